# revision 75
# baseline (speedup 1.0000x reference)
"""BertAttention (B=8, S=1024, H=1024, 16 heads) on 8 TRN2 NeuronCores.

Strategy: data-parallel over batch -- core b computes batch element b
end-to-end (QKV proj, attention, output proj, residual, LayerNorm).
No collectives needed.

Layout notes (per core):
  - All matmul contractions put the contracted dim on SBUF partitions.
  - Host pre-transposes X -> XT [H, S] and weights W -> W^T [in, out]
    so no on-device transposes are needed.
  - QKV projections, the PV matmul, and the output projection all run in
    fp8-e4m3 with MatmulPerfMode.DoubleRow (operands laid out as
    [partitions, 2 contraction-subtiles, cols]; 2x PE throughput).
    Host-quantized: x_q = clip(x*s, +-240), s = 240/absmax (or a
    Cauchy-Schwarz bound for on-device-produced V/ctx).  Dequantization
    is folded downstream, never applied as an extra pass:
      * Q/K scales ride the softmax via the exp() scale operand
        (exp scale = 0.125/(sx^2*swq*swk), DRAM input "qks").
      * V is re-quantized to a power-of-2 scale sv at the projection
        write; the softmax-denominator column then equals exactly 1.0
        and ctx comes out of PV scaled by sv.
      * ctx stays scaled by sv; normalize writes it in the DoubleRow
        pair layout [64, 2, S] (partition-base shift from the PV psum).
        The out-proj psum is then o*(sv*swo); since LayerNorm is
        scale-invariant, the host pre-scales the residual xr by sv*swo
        and ships eps*(sv*swo)^2 ("epsn") -- the scale divides out.
  - Scores run in bf16, transposed: scT[k, q] = K_h Q_h^T so softmax
    denominators come from the scale-column folded into V (PV row 64),
    and exp tiles feed PV directly as the moving operand (fp8, with the
    two k-subtiles of a DoubleRow pair written into one [128, 2, S]
    tile by consecutive exps).
  - Emission is software-pipelined: V projection and next-pair Q/K
    projections interleave into the attention stream, and each PV trails
    its scores by one step so the Activation engine (the exp bottleneck,
    ~113us of the ~205us span) is never starved at pair boundaries.
  - LayerNorm runs on the natural [q, o] output layout (free-dim reduce);
    the (x-mu)*rstd affine runs on the Activation engine (idle in the
    tail) and emits the bf16 output tile directly.
"""

import sys

sys.path.insert(0, "/opt/trn_rl_repo")

import numpy as np

B, S, H = 8, 1024, 1024
NH, HD = 16, 64
NT = 8          # 128-row tiles per 1024 dim
LN_EPS = 1e-12
N_CORES = 8

QKV_FP8 = True
PV_FP8 = True
OP_FP8 = True  # fp8 DoubleRow output projection (requires PV_FP8)

_compiled = {}


def _build(n_reps=1, use_gb=True, use_bv=False, qkv_fp8=QKV_FP8,
           pv_fp8=PV_FP8, op_fp8=OP_FP8):
    assert not op_fp8 or pv_fp8
    import concourse.tile as tile
    from concourse import bacc, mybir

    F32 = mybir.dt.float32
    BF16 = mybir.dt.bfloat16
    F8 = mybir.dt.float8e4
    AF = mybir.ActivationFunctionType
    ALU = mybir.AluOpType
    DR = mybir.MatmulPerfMode.DoubleRow

    DT = BF16  # attention/out-proj matmul dtype

    NT = 8          # 128-row tiles per 1024 dim
    NCH = 2         # 512-col chunks per 1024 dim
    CH = 512

    nc = bacc.Bacc("TRN2", target_bir_lowering=False)

    if qkv_fp8:
        xtp_d = nc.dram_tensor("xtp", [128, 8, S], F8, kind="ExternalInput")
        wqp_d = nc.dram_tensor("wqp", [128, 8, H], F8, kind="ExternalInput")
        wkp_d = nc.dram_tensor("wkp", [128, 8, H], F8, kind="ExternalInput")
        wvp_d = nc.dram_tensor("wvp", [128, 8, H], F8, kind="ExternalInput")
        qks_d = nc.dram_tensor("qks", [128, 1], F32, kind="ExternalInput")
        vsc_d = nc.dram_tensor("vsc", [1, NH], F32, kind="ExternalInput")
        if pv_fp8:
            # V re-quant multiplier (pow2 total V scale / (sx*swv))
            vmul_d = nc.dram_tensor("vmul", [128, 1], F32, kind="ExternalInput")
            if use_bv:
                bvn_d = nc.dram_tensor(
                    "bvn", [64, 2 * NT] if op_fp8 else [128, 8], F32,
                    kind="ExternalInput")
    else:
        xt_d = nc.dram_tensor("xt", [H, S], BF16, kind="ExternalInput")
        wq_d = nc.dram_tensor("wq", [H, H], BF16, kind="ExternalInput")
        wk_d = nc.dram_tensor("wk", [H, H], BF16, kind="ExternalInput")
        wv_d = nc.dram_tensor("wv", [H, H], BF16, kind="ExternalInput")
    if op_fp8:
        # residual is host-prescaled by the out-proj dequant scale (LN is
        # scale-invariant, so the scale divides out exactly; eps ships
        # pre-scaled via "epsn")
        xr_d = nc.dram_tensor("xr", [S, H], BF16, kind="ExternalInput")
        wo8_d = nc.dram_tensor("wo8", [64, 2 * NT, H], F8, kind="ExternalInput")
        epsn_d = nc.dram_tensor("epsn", [128, 1], F32, kind="ExternalInput")
        ident_d = nc.dram_tensor("ident", [128, 128], BF16, kind="ExternalInput")
    else:
        xr_d = nc.dram_tensor("xr", [S, H], F32, kind="ExternalInput")
        wo_d = nc.dram_tensor("wo", [H, H], BF16, kind="ExternalInput")
    bq_d = nc.dram_tensor("bq", [128, 8], F32, kind="ExternalInput")
    bk_d = nc.dram_tensor("bk", [128, 8], F32, kind="ExternalInput")
    bv_d = nc.dram_tensor("bv", [1, H], F32, kind="ExternalInput")
    mask_d = nc.dram_tensor("mask", [128, 8], F32, kind="ExternalInput")
    gamma_d = nc.dram_tensor("gamma", [1, H], F32, kind="ExternalInput")
    beta_d = nc.dram_tensor("beta", [1, H], F32, kind="ExternalInput")
    out_d = nc.dram_tensor("out", [S, H], BF16 if op_fp8 else F32,
                           kind="ExternalOutput")

    with tile.TileContext(nc) as tc:
      for _rep in range(n_reps):
        with (
            tc.tile_pool(name="consts", bufs=1) as cp,
            tc.tile_pool(name="qt", bufs=8) as qt_pool,
            tc.tile_pool(name="kt", bufs=8) as kt_pool,
            tc.tile_pool(name="vt", bufs=(4 if pv_fp8 else 8)) as vt_pool,
            tc.tile_pool(name="ctxp", bufs=8) as ctx_pool,
        ):
            # const tiles (loads are emitted inside the main block, after
            # the big weight DMAs their consumers queue behind)
            bq_sb = cp.tile([128, 8], F32)
            bk_sb = cp.tile([128, 8], F32)
            mask_sb = cp.tile([128, 8], F32)
            eps_sb = cp.tile([128, 1], F32)
            if not op_fp8:
                nc.vector.memset(eps_sb[:], LN_EPS)
            # softmax-denominator column values (V dequant scale; 1.0
            # in the bf16 path)
            ones_sb = cp.tile([128, NH], F32)
            if qkv_fp8:
                vsc_row = cp.tile([1, NH], F32)
                qks_sb = cp.tile([128, 1], F32)
            if pv_fp8:
                vmul_sb = cp.tile([128, 1], F32)
                if use_bv:
                    bvn_sb = cp.tile([64, 2 * NT] if op_fp8 else [128, 8], F32)
            if use_gb:
                gamma_row = cp.tile([1, H], F32)
                beta_row = cp.tile([1, H], F32)
                gamma_sb = cp.tile([128, H], F32)
                beta_sb = cp.tile([128, H], F32)
            if not pv_fp8:
                bv_row = cp.tile([1, H], F32)
                bv_sb = cp.tile([128, H], F32)

            def emit_const_loads():
                nc.sync.dma_start(out=bq_sb, in_=bq_d[:])
                nc.sync.dma_start(out=bk_sb, in_=bk_d[:])
                nc.scalar.dma_start(out=mask_sb, in_=mask_d[:])
                if qkv_fp8:
                    nc.scalar.dma_start(out=vsc_row, in_=vsc_d[:])
                    nc.gpsimd.partition_broadcast(ones_sb[:], vsc_row[:])
                    nc.scalar.dma_start(out=qks_sb, in_=qks_d[:])
                else:
                    nc.vector.memset(ones_sb[:], 1.0)
                if pv_fp8:
                    nc.scalar.dma_start(out=vmul_sb, in_=vmul_d[:])
                    if use_bv:
                        nc.scalar.dma_start(out=bvn_sb, in_=bvn_d[:])
                else:
                    nc.sync.dma_start(out=bv_row, in_=bv_d[:])
                    nc.gpsimd.partition_broadcast(bv_sb[:], bv_row[:])
                if op_fp8:
                    nc.scalar.dma_start(out=eps_sb, in_=epsn_d[:])

            qt = [qt_pool.tile([128, S], DT, tag="qt", name=f"qt{t}") for t in range(NT)]
            kt = [kt_pool.tile([128, S], DT, tag="kt", name=f"kt{t}") for t in range(NT)]
            # v tiles: per k-tile, 16 heads x (64 v-cols + denom col).
            # pv_fp8: consecutive k-tiles are paired in a [128, 2, .] layout
            # for DoubleRow PV matmuls.
            if pv_fp8:
                vt = [vt_pool.tile([128, 2, NH * 65], F8, tag="vt", name=f"vt{t}")
                      for t in range(NT // 2)]
            else:
                vt = [vt_pool.tile([128, NH * 65], DT, tag="vt", name=f"vt{t}")
                      for t in range(NT)]

            # ---- QKV projections + attention, interleaved per head pair ----
            # One shared psum pool for projection tiles and score tiles: no
            # phase barrier, so head pair t's attention (and its exps on the
            # otherwise-idle Activation engine) starts as soon as V and
            # q/k tile t are projected.
            ctxt = []
            with (
                tc.tile_pool(name="xt", bufs=(1 if qkv_fp8 else 8)) as xt_pool,
                tc.tile_pool(name="wp", bufs=(3 if qkv_fp8 else 13)) as wp,
                tc.tile_pool(name="ep", bufs=(8 if pv_fp8 else 12)) as ep,
                tc.tile_pool(name="rp", bufs=4) as rp,
                tc.tile_pool(name="rbp", bufs=3) as rbp,
                tc.tile_pool(name="pp", bufs=(3 if pv_fp8 else 2),
                             space="PSUM") as pp,
                tc.tile_pool(name="cxps", bufs=(2 if pv_fp8 else 4),
                             space="PSUM") as cxps,
            ):
                if qkv_fp8:
                    xtp = xt_pool.tile([128, 8, S], F8, tag="xt", name="xtp")
                    # first half first: the n=0 projection chunks only read
                    # seq columns 0:512
                    nc.scalar.dma_start(out=xtp[:, :, 0:CH],
                                        in_=xtp_d[:, :, 0:CH])
                    nc.scalar.dma_start(out=xtp[:, :, CH:S],
                                        in_=xtp_d[:, :, CH:S])
                    wvp = wp.tile([128, 8, H], F8, tag="w", name="w_v")

                    def qkv_mm(ps_ap, w_sb, n, m, w_stationary):
                        for i in range(4):
                            ko = slice(2 * i, 2 * i + 2)
                            if w_stationary:
                                lhsT = w_sb[:, ko, m * 128:(m + 1) * 128]
                                rhs = xtp[:, ko, n * CH:(n + 1) * CH]
                            else:
                                lhsT = xtp[:, ko, m * 128:(m + 1) * 128]
                                rhs = w_sb[:, ko, n * CH:(n + 1) * CH]
                            nc.tensor.matmul(
                                ps_ap, lhsT=lhsT, rhs=rhs,
                                start=(i == 0), stop=(i == 3),
                                perf_mode=DR,
                            )
                else:
                    xt = []
                    for t in range(NT):
                        x_t = xt_pool.tile([128, S], DT, tag="xt", name=f"xt{t}")
                        (nc.scalar if t % 2 == 0 else nc.gpsimd).dma_start(
                            out=x_t, in_=xt_d[t * 128:(t + 1) * 128, :])
                        xt.append(x_t)
                    wv_tiles = []
                    for t in range(NT):
                        w_t = wp.tile([128, H], DT, tag="w", name=f"w_v{t}")
                        (nc.sync if t % 2 == 0 else nc.scalar).dma_start(
                            out=w_t, in_=wv_d[t * 128:(t + 1) * 128, :])
                        wv_tiles.append(w_t)

                    def qkv_mm(ps_ap, w_tiles_, n, m, w_stationary):
                        for h in range(NT):
                            if w_stationary:
                                lhsT = w_tiles_[h][:, m * 128:(m + 1) * 128]
                                rhs = xt[h][:, n * CH:(n + 1) * CH]
                            else:
                                lhsT = xt[h][:, m * 128:(m + 1) * 128]
                                rhs = w_tiles_[h][:, n * CH:(n + 1) * CH]
                            nc.tensor.matmul(
                                ps_ap, lhsT=lhsT, rhs=rhs,
                                start=(h == 0), stop=(h == NT - 1),
                            )

                wv_sb = wvp if qkv_fp8 else wv_tiles

                # Q/K weights up front.  DMA bandwidth limits the ramp, so
                # load only the 128-col slices pair 0 needs first, then wv
                # (needed by ~5us for the interleaved V projection), then the
                # weight remainders, then the small consts.
                qk_w = {}
                for name, w_dram in (
                    ("q", wqp_d if qkv_fp8 else wq_d),
                    ("k", wkp_d if qkv_fp8 else wk_d),
                ):
                    if qkv_fp8:
                        w_sb = wp.tile([128, 8, H], F8, tag="w", name=f"w_{name}")
                        (nc.sync if name == "q" else nc.gpsimd).dma_start(
                            out=w_sb[:, :, 0:128], in_=w_dram[:, :, 0:128])
                    else:
                        w_sb = []
                        for t in range(NT):
                            w_t = wp.tile([128, H], DT, tag="w", name=f"w_{name}{t}")
                            nc.sync.dma_start(out=w_t, in_=w_dram[t * 128:(t + 1) * 128, :])
                            w_sb.append(w_t)
                    qk_w[name] = w_sb
                emit_const_loads()
                if qkv_fp8:
                    nc.sync.dma_start(out=wvp, in_=wvp_d[:])
                    nc.sync.dma_start(out=qk_w["q"][:, :, 128:H],
                                      in_=wqp_d[:, :, 128:H])
                    nc.gpsimd.dma_start(out=qk_w["k"][:, :, 128:H],
                                        in_=wkp_d[:, :, 128:H])

                def project_qk_part(t, part):
                    # one (name, chunk) quarter of the q/k tile-t projection.
                    # Bias adds read PSUM: DVE only (GPSIMD has no PSUM
                    # access).
                    name, b_sb, dst = (("q", bq_sb, qt) if part < 2
                                       else ("k", bk_sb, kt))
                    n = part % 2
                    ps = pp.tile([128, CH], F32, tag="pp", name="pp_t")
                    qkv_mm(ps[:], qk_w[name], n, t, w_stationary=True)
                    nc.vector.tensor_scalar_add(
                        dst[t][:, n * CH:(n + 1) * CH], ps[:],
                        b_sb[:, t:t + 1],
                    )

                def project_qk(t):
                    # q0,k0 first: the first scores need only the n=0 chunks
                    for part in (0, 2, 1, 3):
                        project_qk_part(t, part)

                def project_v(mk):
                    # V projection: natural [k, dv] layout; x stationary.
                    if pv_fp8:
                        vslab = vt[mk // 2][:, mk % 2, :]
                    else:
                        vslab = vt[mk][:]
                    # denominator columns (V-scale) for the softmax
                    # (SBUF->SBUF: GPSIMD, keeping DVE free for PSUM work)
                    nc.gpsimd.tensor_copy(
                        vslab.rearrange("p (g e) -> p g e", e=65)[:, :, 64:65],
                        ones_sb[:].rearrange("p (g e) -> p g e", e=1),
                    )
                    if pv_fp8:
                        # one [128, S] psum (both 512-chunks as separate
                        # accumulation groups) + one full-width re-quant:
                        # halves the psum-slot pressure of the V burst
                        ps = pp.tile([128, S], F32, tag="pp", name="pv_t")
                        for n in range(NCH):
                            qkv_mm(ps[:, n * CH:(n + 1) * CH], wv_sb, n, mk,
                                   w_stationary=False)
                        nc.vector.tensor_scalar_mul(
                            vslab.rearrange("p (g e) -> p g e", e=65)[:, :, 0:64],
                            ps[:].rearrange("p (g e) -> p g e", e=64),
                            vmul_sb[:],
                        )
                        return
                    for n in range(NCH):
                        ps = pp.tile([128, CH], F32, tag="pp", name="pp_t")
                        qkv_mm(ps[:], wv_sb, n, mk, w_stationary=False)
                        vdst = (vslab[:, n * 8 * 65:(n + 1) * 8 * 65]
                                .rearrange("p (g e) -> p g e", e=65)[:, :, 0:64])
                        nc.vector.tensor_add(
                            vdst,
                            ps[:].rearrange("p (g e) -> p g e", e=64),
                            bv_sb[:, n * CH:(n + 1) * CH]
                            .rearrange("p (g e) -> p g e", e=64),
                        )

                def emit_scores(t, ki, scs_out):
                    # one [128, S] score psum per head per k-tile (2 banks);
                    # the two q-chunks fill its halves; one exp covers both.
                    # Head A (rows 0-63) and head B (rows 64-127) matmuls are
                    # emitted adjacently per chunk: disjoint PE row groups run
                    # concurrently (row tiling).
                    ks = (2 * ki, 2 * ki + 1) if pv_fp8 else (ki,)
                    e_cur = ([ep.tile([128, 2, S], F8, tag="e", name="e_t")
                              for _ in range(2)] if pv_fp8 else [None, None])
                    for ko, k in enumerate(ks):
                        scs = []
                        for hh in range(2):
                            sc = pp.tile([128, S], F32, tag="pp", name="sc_t")
                            scs.append(sc)
                        for n in range(NCH):
                            for hh in range(2):
                                p0 = hh * 64
                                nc.tensor.matmul(
                                    scs[hh][:, n * CH:(n + 1) * CH],
                                    lhsT=kt[t][p0:p0 + 64, k * 128:(k + 1) * 128],
                                    rhs=qt[t][p0:p0 + 64, n * CH:(n + 1) * CH],
                                    start=True,
                                    stop=True,
                                )
                        for hh in range(2):
                            if pv_fp8:
                                # exp straight to fp8 into the ko slot of the
                                # k-pair tile
                                e_dst = e_cur[hh][:, ko, :]
                            else:
                                e_cur[hh] = ep.tile([128, S], DT, tag="e",
                                                    name="e_t")
                                e_dst = e_cur[hh][:]
                            nc.scalar.activation(
                                e_dst, scs[hh][:], AF.Exp,
                                bias=mask_sb[:, k:k + 1],
                                scale=(qks_sb[:] if qkv_fp8 else 0.125),
                            )
                            if not pv_fp8:
                                g = 2 * t + hh
                                for n in range(NCH):
                                    nc.tensor.matmul(
                                        cxs_by_t[t][hh][n][:],
                                        lhsT=vt[k][:, g * 65:(g + 1) * 65],
                                        rhs=e_cur[hh][:, n * CH:(n + 1) * CH],
                                        start=(k == 0),
                                        stop=(k == NT - 1),
                                    )
                    return e_cur

                def emit_pv(t, ki, e_pair):
                    for hh in range(2):
                        g = 2 * t + hh
                        for n in range(NCH):
                            nc.tensor.matmul(
                                cxs_by_t[t][hh][n][:],
                                lhsT=vt[ki][:, :, g * 65:(g + 1) * 65],
                                rhs=e_pair[hh][:, :, n * CH:(n + 1) * CH],
                                start=(ki == 0),
                                stop=(ki == NT // 2 - 1),
                                perf_mode=DR,
                            )

                def emit_normalize(t):
                    ctx_t = ctxt[t]
                    for hh in range(2):
                        for n in range(NCH):
                            cx = cxs_by_t[t][hh][n]
                            recip = rp.tile([1, CH], F32, tag="recip", name="recip_t")
                            nc.vector.reciprocal(recip[:], cx[64:65, :])
                            rb = rbp.tile([64, CH], F32, tag="rb", name="rb_t")
                            nc.gpsimd.partition_broadcast(rb[:], recip[:])
                            if op_fp8:
                                # write fp8 ctx in the DoubleRow pair layout:
                                # partition j, slot hh = feature 64*hh + j of
                                # pair t (ctx is pre-scaled by sv via the
                                # denominator column = 1.0)
                                dst = ctx_t[:, hh, n * CH:(n + 1) * CH]
                            else:
                                dst = ctx_t[hh * 64:hh * 64 + 64,
                                            n * CH:(n + 1) * CH]
                            nc.vector.tensor_mul(dst, cx[0:64, :], rb[:])
                            if pv_fp8 and use_bv:
                                bvs = (bvn_sb[:, 2 * t + hh:2 * t + hh + 1]
                                       if op_fp8 else
                                       bvn_sb[hh * 64:hh * 64 + 64, t:t + 1])
                                nc.vector.tensor_scalar_add(dst, dst, bvs)

                cxs_by_t = {}
                KI = NT // 2 if pv_fp8 else NT

                def begin_pair(t):
                    if op_fp8:
                        ctxt.append(ctx_pool.tile([64, 2, S], F8, tag="ctx",
                                                  name=f"ctx{t}"))
                    else:
                        ctxt.append(ctx_pool.tile([128, S], DT, tag="ctx",
                                                  name=f"ctx{t}"))
                    # 4 live PV accumulators: (head, chunk)
                    cxs_by_t[t] = [[cxps.tile([65, CH], F32, tag="cx",
                                              name="cx_t")
                                    for _ in range(NCH)] for _ in range(2)]

                if not pv_fp8:
                    for mk in range(NT):
                        project_v(mk)
                    project_qk(0)
                    for t in range(NT):
                        begin_pair(t)
                        for ki in range(KI):
                            emit_scores(t, ki, None)
                            if t + 1 < NT and ki == 0:
                                project_qk(t + 1)
                        emit_normalize(t)
                else:
                    # Flat software pipeline over per-HEAD (g, ki) steps.
                    # One [128, S] score psum per step-ko (not two): the pp
                    # pool holds 3 such slots (6 banks) + 2 cx banks = 8, so
                    # the score rotation is 3 deep and the exp stream never
                    # stalls at head/pair boundaries.  Each step emits
                    # scores+exp for step i and the PV matmuls for step i-1.
                    def emit_scores_g(g, ki):
                        t, hh = g // 2, g % 2
                        p0 = hh * 64
                        e_one = ep.tile([128, 2, S], F8, tag="e", name="e_t")
                        for ko in range(2):
                            k = 2 * ki + ko
                            sc = pp.tile([128, S], F32, tag="pp", name="sc_t")
                            for n in range(NCH):
                                nc.tensor.matmul(
                                    sc[:, n * CH:(n + 1) * CH],
                                    lhsT=kt[t][p0:p0 + 64, k * 128:(k + 1) * 128],
                                    rhs=qt[t][p0:p0 + 64, n * CH:(n + 1) * CH],
                                    start=True,
                                    stop=True,
                                )
                            nc.scalar.activation(
                                e_one[:, ko, :], sc[:], AF.Exp,
                                bias=mask_sb[:, k:k + 1],
                                scale=(qks_sb[:] if qkv_fp8 else 0.125),
                            )
                            if g == 0:
                                # fill one v k-tile per ko sub-step (finer
                                # psum-slot spreading than per-step bursts)
                                project_v(k)
                        return e_one

                    def emit_pv_g(g, ki, e_one):
                        for n in range(NCH):
                            nc.tensor.matmul(
                                cxs_by_g[g][n][:],
                                lhsT=vt[ki][:, :, g * 65:(g + 1) * 65],
                                rhs=e_one[:, :, n * CH:(n + 1) * CH],
                                start=(ki == 0),
                                stop=(ki == KI - 1),
                                perf_mode=DR,
                            )

                    def emit_normalize_g(g):
                        t, hh = g // 2, g % 2
                        ctx_t = ctxt[t]
                        for n in range(NCH):
                            cx = cxs_by_g[g][n]
                            recip = rp.tile([1, CH], F32, tag="recip", name="recip_t")
                            nc.vector.reciprocal(recip[:], cx[64:65, :])
                            rb = rbp.tile([64, CH], F32, tag="rb", name="rb_t")
                            nc.gpsimd.partition_broadcast(rb[:], recip[:])
                            if op_fp8:
                                dst = ctx_t[:, hh, n * CH:(n + 1) * CH]
                            else:
                                dst = ctx_t[hh * 64:hh * 64 + 64,
                                            n * CH:(n + 1) * CH]
                            nc.vector.tensor_mul(dst, cx[0:64, :], rb[:])
                            if use_bv:
                                bvs = (bvn_sb[:, g:g + 1] if op_fp8 else
                                       bvn_sb[hh * 64:hh * 64 + 64, t:t + 1])
                                nc.vector.tensor_scalar_add(dst, dst, bvs)

                    cxs_by_g = {}
                    project_qk(0)
                    steps = [(g, ki) for g in range(2 * NT) for ki in range(KI)]
                    pend = None
                    for idx in range(len(steps) + 1):
                        nxt = None
                        if idx < len(steps):
                            g, ki = steps[idx]
                            t, hh = g // 2, g % 2
                            if ki == 0 and hh == 0:
                                if op_fp8:
                                    ctxt.append(ctx_pool.tile(
                                        [64, 2, S], F8, tag="ctx", name=f"ctx{t}"))
                                else:
                                    ctxt.append(ctx_pool.tile(
                                        [128, S], DT, tag="ctx", name=f"ctx{t}"))
                            if ki == 0:
                                cxs_by_g[g] = [cxps.tile([65, CH], F32,
                                                         tag="cx", name="cx_t")
                                               for _ in range(NCH)]
                            e_one = emit_scores_g(g, ki)
                            # next pair's q/k projections, one psum-quarter
                            # per step so they never displace two score slots
                            # at once
                            pt = {(0, 1): 0, (0, 2): 1, (0, 3): 2,
                                  (1, 0): 3}.get((hh, ki))
                            if pt is not None and t + 1 < NT:
                                project_qk_part(t + 1, pt)
                            nxt = (g, ki, e_one)
                        if pend is not None:
                            gp, kip, e_prev = pend
                            emit_pv_g(gp, kip, e_prev)
                            if kip == KI - 1:
                                emit_normalize_g(gp)
                        pend = nxt

            # ---------------- output proj + residual + LayerNorm ----------------
            with (
                tc.tile_pool(name="wo", bufs=(1 if op_fp8 else 8)) as wop,
                tc.tile_pool(name="xr", bufs=5) as xrp,
                tc.tile_pool(name="ob", bufs=4) as obp,
                tc.tile_pool(name="st", bufs=4) as stp,
                tc.tile_pool(name="po", bufs=(8 if op_fp8 else 4),
                             space="PSUM") as po,
            ):
                if op_fp8:
                    wo8_sb = wop.tile([64, 2 * NT, H], F8, tag="wo", name="wo8")
                    nc.sync.dma_start(out=wo8_sb, in_=wo8_d[:])
                    ident_sb = cp.tile([128, 128], BF16)
                    nc.sync.dma_start(out=ident_sb, in_=ident_d[:])
                else:
                    wo_tiles = []
                    for t in range(NT):
                        w_t = wop.tile([128, H], DT, tag="wo", name=f"wo{t}")
                        nc.sync.dma_start(out=w_t, in_=wo_d[t * 128:(t + 1) * 128, :])
                        wo_tiles.append(w_t)
                if use_gb:
                    nc.sync.dma_start(out=gamma_row, in_=gamma_d[:])
                    nc.sync.dma_start(out=beta_row, in_=beta_d[:])
                    nc.gpsimd.partition_broadcast(gamma_sb[:], gamma_row[:])
                    nc.gpsimd.partition_broadcast(beta_sb[:], beta_row[:])
                for mq in range(NT):
                    xr_t = xrp.tile([128, H], BF16 if op_fp8 else F32,
                                    tag="xr", name="xr_t")
                    (nc.sync if mq % 2 == 0 else nc.gpsimd).dma_start(
                        out=xr_t, in_=xr_d[mq * 128:(mq + 1) * 128, :]
                    )
                    stats = stp.tile([128, 2, 6], F32, tag="stats", name="stats_t")
                    pss = []
                    if not op_fp8:
                        o_t = obp.tile([128, H], F32, tag="ob", name="ob_t")
                    for n in range(NCH):
                        ps = po.tile([128, CH], F32, tag="po", name="po_t")
                        pss.append(ps)
                        for t in range(NT):
                            if op_fp8:
                                nc.tensor.matmul(
                                    ps[:],
                                    lhsT=ctxt[t][:, :, mq * 128:(mq + 1) * 128],
                                    rhs=wo8_sb[:, 2 * t:2 * t + 2,
                                               n * CH:(n + 1) * CH],
                                    start=(t == 0),
                                    stop=False,
                                    perf_mode=DR,
                                )
                            else:
                                nc.tensor.matmul(
                                    ps[:],
                                    lhsT=ctxt[t][:, mq * 128:(mq + 1) * 128],
                                    rhs=wo_tiles[t][:, n * CH:(n + 1) * CH],
                                    start=(t == 0),
                                    stop=(t == NT - 1),
                                )
                        if op_fp8:
                            # residual add via identity matmul into the
                            # accumulating psum: no DVE pass over the tile
                            nc.tensor.matmul(
                                ps[:], lhsT=ident_sb[:],
                                rhs=xr_t[:, n * CH:(n + 1) * CH],
                                start=False, stop=True,
                            )
                            nc.vector.bn_stats(stats[:, n, :], ps[:])
                        else:
                            nc.vector.tensor_add(
                                o_t[:, n * CH:(n + 1) * CH], ps[:],
                                xr_t[:, n * CH:(n + 1) * CH],
                            )
                            nc.vector.bn_stats(stats[:, n, :],
                                               o_t[:, n * CH:(n + 1) * CH])
                    mv = stp.tile([128, 2], F32, tag="mv", name="mv_t")
                    nc.vector.bn_aggr(mv[:], stats[:])
                    mu = mv[:, 0:1]
                    var = mv[:, 1:2]
                    std = stp.tile([128, 1], F32, tag="std", name="std_t")
                    nc.scalar.activation(std[:], var[:], AF.Sqrt, bias=eps_sb[:])
                    rstd = stp.tile([128, 1], F32, tag="rstd", name="rstd_t")
                    nc.vector.reciprocal(rstd[:], std[:])
                    # (x - mu) * rstd affine: rstd*x + (-mu*rstd)
                    nmur = stp.tile([128, 1], F32, tag="nmur", name="nmur_t")
                    nc.vector.tensor_scalar(
                        out=nmur[:], in0=mu, scalar1=rstd[:], scalar2=-1.0,
                        op0=ALU.mult, op1=ALU.mult,
                    )
                    o_fin = obp.tile([128, H], BF16 if op_fp8 else F32,
                                     tag="of", name="of_t")
                    if use_gb:
                        o_g = obp.tile([128, H], F32, tag="og", name="og_t")
                        if op_fp8:
                            for n in range(NCH):
                                nc.scalar.activation(
                                    o_g[:, n * CH:(n + 1) * CH], pss[n][:],
                                    AF.Identity, bias=nmur[:], scale=rstd[:],
                                )
                        else:
                            nc.scalar.activation(
                                o_g[:], o_t[:], AF.Identity,
                                bias=nmur[:], scale=rstd[:],
                            )
                        nc.vector.tensor_mul(o_g[:], o_g[:], gamma_sb[:])
                        nc.vector.tensor_add(o_fin[:], o_g[:], beta_sb[:])
                    elif op_fp8:
                        # affine on the Activation engine (idle in the tail),
                        # reading the psum chunks directly
                        for n in range(NCH):
                            nc.scalar.activation(
                                o_fin[:, n * CH:(n + 1) * CH], pss[n][:],
                                AF.Identity, bias=nmur[:], scale=rstd[:],
                            )
                    else:
                        nc.scalar.activation(
                            o_fin[:], o_t[:], AF.Identity,
                            bias=nmur[:], scale=rstd[:],
                        )
                    (nc.gpsimd if mq % 2 == 0 else nc.sync).dma_start(
                        out=out_d[mq * 128:(mq + 1) * 128, :], in_=o_fin
                    )

    nc.compile()
    return nc


def _q8(a, s):
    import ml_dtypes
    return np.clip(a * s, -240.0, 240.0).astype(ml_dtypes.float8_e4m3)


def _fold_pairs(wT):
    # [K, M] -> [128, K//128, M]: partition j, subtile t = row 128*t+j
    K, M = wT.shape
    return np.ascontiguousarray(wT.reshape(K // 128, 128, M).transpose(1, 0, 2))


def _host_prep(hidden_states, attention_mask, Wq, bq, Wk, bk, Wv, bv,
               Wo, bo, ln_gamma, ln_beta, qkv_fp8=QKV_FP8, pv_fp8=PV_FP8,
               op_fp8=OP_FP8):
    import ml_dtypes
    f32 = np.float32
    bf16 = ml_dtypes.bfloat16
    hs = np.ascontiguousarray(hidden_states, dtype=f32)
    wqT = np.ascontiguousarray(np.asarray(Wq, dtype=f32).T)
    wkT = np.ascontiguousarray(np.asarray(Wk, dtype=f32).T)
    wvT = np.ascontiguousarray(np.asarray(Wv, dtype=f32).T)
    woT = np.ascontiguousarray(np.asarray(Wo, dtype=f32).T)
    gamma_r = np.ascontiguousarray(np.asarray(ln_gamma, f32).reshape(1, H))
    beta_r = np.ascontiguousarray(np.asarray(ln_beta, f32).reshape(1, H))
    bo_r = np.asarray(bo, f32)
    mask = np.asarray(attention_mask, f32).reshape(B, S)
    bq_ = np.asarray(bq, f32)
    bk_ = np.asarray(bk, f32)
    bv_ = np.asarray(bv, f32)

    if qkv_fp8:
        swq = 240.0 / max(np.abs(wqT).max(), 1e-30)
        swk = 240.0 / max(np.abs(wkT).max(), 1e-30)
        swv = 240.0 / max(np.abs(wvT).max(), 1e-30)
        wqp = _q8(_fold_pairs(wqT), swq)
        wkp = _q8(_fold_pairs(wkT), swk)
        wvp = _q8(_fold_pairs(wvT), swv)
    if pv_fp8:
        # Safe upper bound on |v| = |x @ Wv^T + bv| via Cauchy-Schwarz;
        # total V scale is a power of two so the fp8 denominator column
        # (which must equal sv/sc8 exactly) is exactly representable.
        # With op_fp8, ctx is kept scaled by sv (|ctx| <= vbound too), so
        # the denominator column is exactly 1.0.
        wvn = float(np.linalg.norm(wvT, axis=0).max())
        bvmax = float(np.abs(bv_).max())
    if op_fp8:
        swo = 240.0 / max(np.abs(woT).max(), 1e-30)
        # wo8[j, 2t+hh, o] = woT[128t + 64hh + j, o] * swo
        wo8 = _q8(np.ascontiguousarray(
            woT.reshape(NT, 2, 64, H).transpose(2, 0, 1, 3)
            .reshape(64, 2 * NT, H)), swo)
    else:
        woT_bf = woT.astype(bf16)

    in_maps = []
    for b in range(B):
        mask_r = np.ascontiguousarray(mask[b].reshape(8, 128).T)
        m = {
            "mask": mask_r, "gamma": gamma_r, "beta": beta_r,
        }
        xT = np.ascontiguousarray(hs[b].T)
        if qkv_fp8:
            sx = 240.0 / max(np.abs(xT).max(), 1e-30)
            m["xtp"] = _q8(_fold_pairs(xT), sx)
            m["wqp"], m["wkp"], m["wvp"] = wqp, wkp, wvp
            m["qks"] = np.full((128, 1), 0.125 / (sx * sx * swq * swk), f32)
            m["bq"] = np.ascontiguousarray(
                (bq_ * (sx * swq)).reshape(8, 128).T)
            m["bk"] = np.ascontiguousarray(
                (bk_ * (sx * swk)).reshape(8, 128).T)
        else:
            m["xt"] = xT.astype(bf16)
            m["wq"], m["wk"], m["wv"] = (
                wqT.astype(bf16), wkT.astype(bf16), wvT.astype(bf16))
            m["bq"] = np.ascontiguousarray(bq_.reshape(8, 128).T)
            m["bk"] = np.ascontiguousarray(bk_.reshape(8, 128).T)
        if pv_fp8:
            xn = float(np.linalg.norm(hs[b], axis=1).max())
            vbound = max(xn * wvn + bvmax, 1e-30)
            sv = 2.0 ** np.floor(np.log2(240.0 / vbound))
            m["vmul"] = np.full(
                (128, 1), sv / ((sx * swv) if qkv_fp8 else 1.0), f32)
            m["bv"] = np.ascontiguousarray(bv_.reshape(1, H))
            if op_fp8:
                m["vsc"] = np.full((1, NH), 1.0, f32)
                m["bvn"] = np.ascontiguousarray(
                    (bv_ * sv).reshape(NT, 2, 64).transpose(2, 0, 1)
                    .reshape(64, 2 * NT))
            else:
                m["vsc"] = np.full((1, NH), sv, f32)
                m["bvn"] = np.ascontiguousarray(bv_.reshape(8, 128).T)
        elif qkv_fp8:
            m["vsc"] = np.full((1, NH), sx * swv, f32)
            m["bv"] = np.ascontiguousarray(
                (bv_ * (sx * swv)).reshape(1, H))
        else:
            m["bv"] = np.ascontiguousarray(bv_.reshape(1, H))
        if op_fp8:
            c = sv * swo
            m["xr"] = ((hs[b] + bo_r[None, :]) * c).astype(bf16)
            m["wo8"] = wo8
            m["epsn"] = np.full((128, 1), LN_EPS * c * c, f32)
            m["ident"] = np.eye(128, dtype=f32).astype(bf16)
        else:
            m["xr"] = np.ascontiguousarray(hs[b] + bo_r[None, :])
            m["wo"] = woT_bf
        in_maps.append(m)
    return in_maps


def get_nc(n_reps=1, use_gb=True, use_bv=False, qkv_fp8=QKV_FP8,
           pv_fp8=PV_FP8, op_fp8=OP_FP8):
    key = (n_reps, use_gb, use_bv, qkv_fp8, pv_fp8, op_fp8)
    if key not in _compiled:
        _compiled[key] = _build(n_reps, use_gb, use_bv, qkv_fp8, pv_fp8,
                                op_fp8)
    return _compiled[key]


def kernel(hidden_states, attention_mask, Wq, bq, Wk, bk, Wv, bv, Wo, bo,
           ln_gamma, ln_beta):
    from concourse.bass_utils import run_bass_kernel_spmd

    use_gb = not (
        np.all(np.asarray(ln_gamma) == 1.0) and np.all(np.asarray(ln_beta) == 0.0)
    )
    use_bv = bool(np.any(np.asarray(bv) != 0.0))
    nc = get_nc(use_gb=use_gb, use_bv=use_bv)
    in_maps = _host_prep(hidden_states, attention_mask, Wq, bq,
                         Wk, bk, Wv, bv, Wo, bo, ln_gamma, ln_beta)
    res = run_bass_kernel_spmd(nc, in_maps, list(range(N_CORES)))
    out = np.stack([np.asarray(res.results[i]["out"]) for i in range(N_CORES)])
    return out.astype(np.float32)
